# revision 14
# baseline (speedup 1.0000x reference)
"""MultiHeadAttention Trainium2 Bass kernel.

Head-sharded tensor parallel across 8 NeuronCores (2 heads/core).
All-transposed dataflow: activations live feature-on-partition so no
on-device activation transposes are needed; the per-head attention
computes S.T = K Q.T directly, softmax is max-free (scores are bounded),
the additive attention bias is applied as a multiply by exp(bias)
(precomputed on host), and the key-padding mask is applied by zeroing
masked v rows + masking the denominator matmul.

Host side: inputs are pre-transposed / pre-cast to fp16, outputs are
partial sums (row-parallel out projection) summed on host.
"""

import sys

sys.path.insert(0, "/opt/trn_rl_repo")

import numpy as np

B, S, H, NH = 2, 2048, 1024, 16
HD = H // NH            # 64
NCORES = 8
HPC = NH // NCORES      # 2 heads per core
CW = HPC * HD           # 128 = per-core slice width
R = B * S               # 4096 flattened rows
SCALE = float(HD) ** -0.5
F = H // 128            # 8 feature blocks
RC = R // 512           # 8 row chunks
QC = S // 512           # 4 q chunks per batch
KB = S // 128           # 16 k blocks per batch
T = B * KB              # 32 (b, kb) blocks

_CACHE = {}


def _build_module():
    import concourse.bass as bass
    import concourse.tile as tile
    from concourse import bacc, mybir
    from concourse.masks import make_identity

    f16 = mybir.dt.float16
    f32 = mybir.dt.float32
    Exp = mybir.ActivationFunctionType.Exp

    nc = bacc.Bacc(
        "TRN2", target_bir_lowering=False, debug=False, num_devices=NCORES
    )

    # ---- DRAM I/O (per core) ----
    xq = nc.dram_tensor("xq_t", [H, R], f16, kind="ExternalInput").ap()
    xk = nc.dram_tensor("xk_t", [H, R], f16, kind="ExternalInput").ap()
    xv = nc.dram_tensor("xv_t", [H, R], f16, kind="ExternalInput").ap()
    wq = nc.dram_tensor("wq_t", [H, CW], f16, kind="ExternalInput").ap()
    wk = nc.dram_tensor("wk_t", [H, CW], f16, kind="ExternalInput").ap()
    wv = nc.dram_tensor("wv_t", [H, CW], f16, kind="ExternalInput").ap()
    wo = nc.dram_tensor("wo_t", [CW, H], f16, kind="ExternalInput").ap()
    qb = nc.dram_tensor("qb_col", [CW, 1], f32, kind="ExternalInput").ap()
    kb_ = nc.dram_tensor("kb_col", [CW, 1], f32, kind="ExternalInput").ap()
    eb = nc.dram_tensor("eb_t", [HPC, S, S], f16, kind="ExternalInput").ap()
    m01f = nc.dram_tensor("m01_f32", [128, T], f32, kind="ExternalInput").ap()
    m01h = nc.dram_tensor("m01_v", [128, T], f16, kind="ExternalInput").ap()
    opart = nc.dram_tensor("o_part", [R, H], f16, kind="ExternalOutput").ap()

    with tile.TileContext(nc) as tc:
        _emit(tc, nc, f16, f32, Exp, make_identity, bass,
              xq, xk, xv, wq, wk, wv, wo, qb, kb_, eb, m01f, m01h, opart)

    nc.compile()
    return nc


def _emit(tc, nc, f16, f32, Exp, make_identity, bass,
          xq, xk, xv, wq, wk, wv, wo, qb, kb_, eb, m01f, m01h, opart):
    from contextlib import ExitStack

    with ExitStack() as top:
        consts = top.enter_context(tc.tile_pool(name="consts", bufs=1))
        pers = top.enter_context(tc.tile_pool(name="pers", bufs=1))
        xpool = top.enter_context(tc.tile_pool(name="xin", bufs=6))

        # ---- constants / weights resident in SBUF ----
        wq_sb = consts.tile([128, F, 128], f16, tag="wq")
        nc.sync.dma_start(wq_sb, wq.rearrange("(f p) j -> p f j", p=128))
        wk_sb = consts.tile([128, F, 128], f16, tag="wk")
        nc.sync.dma_start(wk_sb, wk.rearrange("(f p) j -> p f j", p=128))
        wv_sb = consts.tile([128, F, 128], f16, tag="wv")
        nc.sync.dma_start(wv_sb, wv.rearrange("(f p) j -> p f j", p=128))
        wo_sb = consts.tile([128, H], f16, tag="wo")
        nc.sync.dma_start(wo_sb, wo)
        qb_sb = consts.tile([128, 1], f32, tag="qb")
        nc.sync.dma_start(qb_sb, qb)
        kb_sb = consts.tile([128, 1], f32, tag="kb")
        nc.sync.dma_start(kb_sb, kb_)
        m01f_sb = consts.tile([128, T], f32, tag="m01f")
        nc.sync.dma_start(m01f_sb, m01f)
        ident = consts.tile([128, 128], f16, tag="ident")
        make_identity(nc, ident)

        # ---- persistent activations ----
        qT_sb = pers.tile([128, R], f16, tag="qT")     # [2h*64d, (b,s)]
        kT_sb = pers.tile([128, R], f16, tag="kT")
        # v_aug layout per (b,kb) block t: [v_h0 (0:64) | m01 (64) | pad |
        #                                   v_h1 (66:130) | m01 (130) | pad]
        v_nat = pers.tile([128, T, 132], f16, tag="vn")
        # fill the mask columns (the PV "ones column" → masked denominator)
        nc.sync.dma_start(v_nat[:, :, 64:65], m01h)
        nc.sync.dma_start(v_nat[:, :, 130:131], m01h)
        ctxn = [pers.tile([128, S], f16, tag=f"ctxn{b}", name=f"ctxn{b}")
                for b in range(B)]

        # =================== phase 1: projections ===================
        with tc.tile_pool(name="p1psum", bufs=4, space="PSUM") as p1, \
             tc.tile_pool(name="ptrans", bufs=3, space="PSUM") as ptr, \
             tc.tile_pool(name="vt", bufs=2) as vtp:

            for w_sb, x_dram, dst, bias_col in (
                (wq_sb, xq, qT_sb, qb_sb),
                (wk_sb, xk, kT_sb, kb_sb),
            ):
                for rc in range(RC):
                    ps = p1.tile([128, 512], f32, tag="p1")
                    for f in range(F):
                        xt = xpool.tile([128, 512], f16, tag="xt")
                        nc.sync.dma_start(
                            xt, x_dram[f * 128:(f + 1) * 128,
                                       rc * 512:(rc + 1) * 512])
                        nc.tensor.matmul(ps, lhsT=w_sb[:, f, :], rhs=xt,
                                         start=(f == 0), stop=(f == F - 1))
                    nc.vector.tensor_scalar_add(
                        dst[:, rc * 512:(rc + 1) * 512], ps, bias_col)

            # v: project (v.T chunks), then PE-transpose to natural layout,
            # zeroing masked key rows via the 0/1 mask column.
            for rc in range(RC):
                ps = p1.tile([128, 512], f32, tag="p1")
                for f in range(F):
                    xt = xpool.tile([128, 512], f16, tag="xt")
                    nc.sync.dma_start(
                        xt, xv[f * 128:(f + 1) * 128, rc * 512:(rc + 1) * 512])
                    nc.tensor.matmul(ps, lhsT=wv_sb[:, f, :], rhs=xt,
                                     start=(f == 0), stop=(f == F - 1))
                vt = vtp.tile([128, 512], f16, tag="vt")
                nc.vector.tensor_copy(vt, ps)
                for i in range(4):
                    t = rc * 4 + i          # t = b*KB + kb
                    col = (t % KB) * B + t // KB
                    tp = ptr.tile([128, 128], f16, tag="tp")
                    nc.tensor.transpose(tp, vt[:, i * 128:(i + 1) * 128], ident)
                    for h in range(HPC):
                        nc.vector.tensor_scalar_mul(
                            v_nat[:, t, h * 66:h * 66 + 64],
                            tp[:, h * 64:(h + 1) * 64],
                            m01f_sb[:, col:col + 1])

        # =================== phase 2: attention ===================
        with tc.tile_pool(name="qkpsum", bufs=2, space="PSUM") as qkp, \
             tc.tile_pool(name="cvpsum", bufs=4, space="PSUM") as cvp_pool, \
             tc.tile_pool(name="ebp", bufs=3) as ebp, \
             tc.tile_pool(name="esp", bufs=3) as esp, \
             tc.tile_pool(name="ptp", bufs=3) as ptp, \
             tc.tile_pool(name="bcp", bufs=4) as bcp, \
             tc.tile_pool(name="h1p", bufs=4) as h1p, \
             tc.tile_pool(name="rcp", bufs=2) as rcp, \
             tc.tile_pool(name="dscr", bufs=4, space="DRAM") as dscr:

            for qc in range(QC):
                cvp = [[cvp_pool.tile([65, 512], f32, tag="cv",
                                      name=f"cv{qc}_{b}_{h}")
                        for h in range(HPC)] for b in range(B)]

                for kb in range(KB):
                    ebt = ebp.tile([128, HPC, 512], f16, tag="eb")
                    nc.sync.dma_start(
                        ebt,
                        eb[:, kb * 128:(kb + 1) * 128,
                           qc * 512:(qc + 1) * 512].rearrange("i p j -> p i j"))

                    sps, es, pt = [], [], []
                    for b in range(B):
                        sps.append(qkp.tile([128, HPC, 512], f32, tag="sps",
                                            name=f"sps{qc}_{kb}_{b}"))
                        for h in range(HPC):
                            nc.tensor.matmul(
                                sps[b][:, h, :],
                                lhsT=kT_sb[h * 64:(h + 1) * 64,
                                           b * S + kb * 128:
                                           b * S + (kb + 1) * 128],
                                rhs=qT_sb[h * 64:(h + 1) * 64,
                                          b * S + qc * 512:
                                          b * S + (qc + 1) * 512],
                                start=True, stop=True)
                    for b in range(B):
                        est = esp.tile([128, HPC, 512], f16, tag="es")
                        nc.scalar.activation(est, sps[b], func=Exp, scale=SCALE)
                        es.append(est)
                    for b in range(B):
                        ptt = ptp.tile([128, HPC, 512], f16, tag="pt")
                        eng = nc.vector if b == 0 else nc.gpsimd
                        eng.tensor_mul(ptt, es[b], ebt)
                        pt.append(ptt)

                    first = kb == 0
                    last = kb == KB - 1
                    for b in range(B):
                        for h in range(HPC):
                            # v_aug lhsT: 64 v cols + the 0/1 mask column →
                            # rows 0-63 = ctx.T, row 64 = masked denominator
                            nc.tensor.matmul(
                                cvp[b][h],
                                lhsT=v_nat[:, b * KB + kb,
                                           h * 66:h * 66 + 65],
                                rhs=pt[b][:, h, :],
                                start=first, stop=last)

                # normalize: ctxn = ctx.T * (1/den), per (b, h)
                for b in range(B):
                    for h in range(HPC):
                        rc_sb = rcp.tile([65, 512], f32, tag="rc")
                        nc.vector.reciprocal(rc_sb[64:65, :],
                                             cvp[b][h][64:65, :])
                        scr = dscr.tile([1, 512], f32, tag="scr",
                                        name=f"scr{qc}_{b}_{h}")
                        nc.sync.dma_start(scr, rc_sb[64:65, :])
                        bc = bcp.tile([64, 512], f32, tag="bc")
                        nc.sync.dma_start(bc, scr.to_broadcast((64, 512)))
                        if h == 0:
                            nc.vector.tensor_mul(
                                ctxn[b][0:64, qc * 512:(qc + 1) * 512],
                                cvp[b][h][0:64, :], bc)
                        else:
                            # lanes 0-63 → relocate to partitions 64-127
                            h1t = h1p.tile([64, 512], f16, tag="h1t")
                            nc.vector.tensor_mul(h1t, cvp[b][h][0:64, :], bc)
                            nc.sync.dma_start(
                                ctxn[b][64:128, qc * 512:(qc + 1) * 512],
                                h1t)

        # =================== phase 3: out projection ===================
        with tc.tile_pool(name="p3psum", bufs=2, space="PSUM") as p3, \
             tc.tile_pool(name="op", bufs=3) as op:
            for b in range(B):
                for rb in range(S // 128):
                    po = p3.tile([128, 2, 512], f32, tag="po")
                    lhsT = ctxn[b][:, rb * 128:(rb + 1) * 128]
                    nc.tensor.matmul(po[:, 0, :], lhsT=lhsT,
                                     rhs=wo_sb[:, 0:512], start=True, stop=True)
                    nc.tensor.matmul(po[:, 1, :], lhsT=lhsT,
                                     rhs=wo_sb[:, 512:1024], start=True,
                                     stop=True)
                    ob = op.tile([128, 2, 512], f16, tag="ob")
                    nc.vector.tensor_copy(ob, po)
                    r0 = (b * (S // 128) + rb) * 128
                    nc.sync.dma_start(
                        opart[r0:r0 + 128, :],
                        ob.rearrange("p i j -> p (i j)"))


def get_module():
    if "nc" not in _CACHE:
        _CACHE["nc"] = _build_module()
    return _CACHE["nc"]


def make_in_maps(query, key, value, key_padding_mask, bias,
                 q_w, q_b, k_w, k_b, v_w, v_b, o_w, o_b):
    f16 = np.float16
    xq_t = np.ascontiguousarray(query.reshape(R, H).T).astype(f16)
    xk_t = np.ascontiguousarray(key.reshape(R, H).T).astype(f16)
    xv_t = np.ascontiguousarray(value.reshape(R, H).T).astype(f16)

    kpm = np.asarray(key_padding_mask)
    # m01[p, b*?]: column index kb*B + b ; 0.0 where masked
    m01 = np.empty((128, T), np.float32)
    for b in range(B):
        for kb in range(KB):
            m01[:, kb * B + b] = np.where(kpm[b, kb * 128:(kb + 1) * 128],
                                          0.0, 1.0)
    m01_f32 = np.ascontiguousarray(m01)
    # v-order mask: column t = b*KB + kb (matches the v_nat block order)
    m01v = np.empty((128, T), f16)
    for b in range(B):
        for kb in range(KB):
            m01v[:, b * KB + kb] = m01[:, kb * B + b].astype(f16)

    in_maps = []
    for c in range(NCORES):
        hs = slice(c * CW, (c + 1) * CW)
        ebt = np.empty((HPC, S, S), f16)
        for i in range(HPC):
            h = c * HPC + i
            ebt[i] = np.exp(np.asarray(bias[0, h], np.float32).T).astype(f16)
        in_maps.append({
            "xq_t": xq_t, "xk_t": xk_t, "xv_t": xv_t,
            "wq_t": np.ascontiguousarray(np.asarray(q_w)[hs].T).astype(f16),
            "wk_t": np.ascontiguousarray(np.asarray(k_w)[hs].T).astype(f16),
            "wv_t": np.ascontiguousarray(np.asarray(v_w)[hs].T).astype(f16),
            "wo_t": np.ascontiguousarray(np.asarray(o_w)[:, hs].T).astype(f16),
            "qb_col": np.asarray(q_b, np.float32)[hs].reshape(CW, 1).copy(),
            "kb_col": np.asarray(k_b, np.float32)[hs].reshape(CW, 1).copy(),
            "eb_t": ebt,
            "m01_f32": m01_f32,
            "m01_v": m01v,
        })
    return in_maps


def assemble_output(results, v_b, o_w, o_b):
    acc = np.zeros((R, H), np.float32)
    for res in results:
        acc += np.asarray(res["o_part"], np.float32)
    corr = np.asarray(v_b, np.float32) @ np.asarray(o_w, np.float32).T \
        + np.asarray(o_b, np.float32)
    acc += corr[None, :]
    return acc.reshape(B, S, H).astype(np.float32)


def kernel(**inputs):
    from concourse.bass_utils import run_bass_kernel_spmd

    nc = get_module()
    in_maps = make_in_maps(**inputs)
    res = run_bass_kernel_spmd(nc, in_maps, list(range(NCORES)))
    return assemble_output(res.results, inputs["v_b"], inputs["o_w"],
                           inputs["o_b"])
